# revision 9
# baseline (speedup 1.0000x reference)
"""Trainium2 (Bass/Tile) kernel for the ClusterMemory problem.

Computes, for full inputs:
  x = normalize(inputs)                      [B, D]
  logits = x @ features.T / TEMP             [B, N]
  loss = cross_entropy(logits, targets)      scalar
  new_features = sequential per-sample momentum update + renorm of bank rows

Sharding: the feature bank [N, D] is sharded row-wise across 8 NeuronCores.
Each core streams its shard once (HBM read), computes partial logits via the
tensor engine (on-chip PE transpose of the shard, fp32r matmuls), reduces
exp(logits) row-wise on the scalar engine, and passes the shard through to the
output bank.  The per-target momentum update (a scatter over <=256 rows) is
done on the owning core with indirect-DMA gathers of the affected rows +
masked chain updates (handles duplicate targets sequentially), then an
indirect-DMA scatter into the output bank, ordered after the pass-through
writes.  The host combines per-core partial softmax statistics (sum-exp and
target logits) into the scalar loss and concatenates the output shards.
"""

import os
import numpy as np

B, D, N = 256, 2048, 30000
NCORES = 8
NS = N // NCORES  # 3750 rows per core
TEMP = 0.05
ITEMP = 1.0 / TEMP
MOM = 0.2
OMM = 1.0 - MOM
EPS = 1e-12
P = 128
SUB = 4  # 128-row subtiles per streaming block

_LAST_RESULTS = {}


def _schedule(targets):
    """Per-core chains: for core c, sorted list of (local_row, [sample indices])."""
    percore = []
    t_max, r_max = 1, 1
    tl = [int(t) for t in targets]
    for c in range(NCORES):
        lo = c * NS
        rows = {}
        for i, t in enumerate(tl):
            if lo <= t < lo + NS:
                rows.setdefault(t - lo, []).append(i)
        chains = sorted(rows.items())
        percore.append(chains)
        if chains:
            t_max = max(t_max, max(len(s) for _, s in chains))
            r_max = max(r_max, len(chains))
    rq = max(1, -(-r_max // P))
    return percore, t_max, rq


def _build(T_MAX, RQ):
    VAR = os.environ.get("KVAR", "full")
    import concourse.bass as bass
    import concourse.tile as tile
    from concourse import bacc, mybir
    from concourse.masks import make_identity

    f32 = mybir.dt.float32
    f32r = mybir.dt.float32r
    i32 = mybir.dt.int32
    AL = mybir.AluOpType
    ACTF = mybir.ActivationFunctionType
    AX = mybir.AxisListType

    nc = bacc.Bacc(None, target_bir_lowering=False, debug=True)
    ST = RQ * T_MAX + 2

    xin = nc.dram_tensor("xin", [B, D], f32, kind="ExternalInput")
    fsh = nc.dram_tensor("fshard", [NS, D], f32, kind="ExternalInput")
    gidx_d = nc.dram_tensor("gidx", [RQ, P, 1], i32, kind="ExternalInput")
    sidx_d = nc.dram_tensor("sidx", [RQ, P, 1], i32, kind="ExternalInput")
    gsix_d = nc.dram_tensor("gsidx", [RQ, T_MAX, P, 1], i32, kind="ExternalInput")
    cofs_d = nc.dram_tensor("coefs", [RQ, P, 4 * T_MAX], f32, kind="ExternalInput")

    obank = nc.dram_tensor("out_bank", [NS, D], f32, kind="ExternalOutput")
    ostat = nc.dram_tensor("out_stats", [ST, P], f32, kind="ExternalOutput")

    # streaming structure: 29 full 128-row subtiles + one 38-row tail
    nfull = NS // P
    rem = NS - nfull * P
    subs = [(j * P, P) for j in range(nfull)]
    if rem:
        subs.append((nfull * P, rem))
    if VAR == "notail":
        subs = subs[:28]
    blocks = [subs[i:i + SUB] for i in range(0, len(subs), SUB)]
    NB = len(blocks)

    with tile.TileContext(nc) as tc, \
         tc.tile_pool(name="const", bufs=1) as constp, \
         tc.tile_pool(name="persist", bufs=1) as persist, \
         tc.tile_pool(name="small", bufs=1) as small, \
         tc.tile_pool(name="gx", bufs=2) as gxp, \
         tc.tile_pool(name="nat", bufs=2) as natp, \
         tc.tile_pool(name="rhsT", bufs=2) as rtp, \
         tc.tile_pool(name="psT", bufs=2, space="PSUM") as psTp, \
         tc.tile_pool(name="psM", bufs=2, space="PSUM") as psMp, \
         tc.tile_pool(name="esc", bufs=2) as escp:

        ident = constp.tile([P, P], f32)
        make_identity(nc, ident)

        xT = persist.tile([P, 16 * B], f32)      # [d(128), k2*256 + sample]
        sums = persist.tile([P, 2 * NB], f32)    # per-block exp-sums
        st = persist.tile([P, ST], f32)          # stats: tlogits + sumexp halves
        if VAR in ("noc", "notail"):
            nc.vector.memset(st[:], 0.0)
        scr = persist.tile([P, D], f32)          # elementwise scratch

        # ---- Stage A/B: normalize x, transpose to xT ----
        with tc.tile_pool(name="xtmp", bufs=1) as xtmp, \
             tc.tile_pool(name="psX", bufs=2, space="PSUM") as psXp:
            xraw = xtmp.tile([P, 2, D], f32)
            nc.sync.dma_start(out=xraw[:], in_=xin[:].rearrange("(k p) d -> p k d", p=P))
            ss = small.tile([P, 4], f32, tag="ssA")
            for k in range(2):
                nc.scalar.activation(out=scr[:], in_=xraw[:, k, :],
                                     func=ACTF.Square, accum_out=ss[:, 0:1])
                nc.scalar.sqrt(ss[:, 1:2], ss[:, 0:1])
                nc.vector.tensor_scalar_add(ss[:, 1:2], ss[:, 1:2], EPS)
                nc.vector.reciprocal(ss[:, 2:3], ss[:, 1:2])
                nc.vector.tensor_scalar_mul(xraw[:, k, :], xraw[:, k, :], ss[:, 2:3])
            for k2 in range(16):
                psx = psXp.tile([P, B], f32, tag="psX")
                for k in range(2):
                    nc.tensor.transpose(
                        out=psx[:, k * P:(k + 1) * P],
                        in_=xraw[:, k, k2 * P:(k2 + 1) * P],
                        identity=ident[:])
                if k2 % 2:
                    nc.scalar.copy(xT[:, k2 * B:(k2 + 1) * B].bitcast(f32r), psx[:])
                else:
                    nc.vector.tensor_copy(xT[:, k2 * B:(k2 + 1) * B].bitcast(f32r), psx[:])

        # ---- Stage C: gather target rows, chained momentum update ----
        gtiles, sitiles = [], []
        for q in range(RQ if VAR not in ("noc", "notail") else 0):
            gi = small.tile([P, 1], i32, tag=f"gi{q}")
            si = small.tile([P, 1], i32, tag=f"si{q}")
            cf = small.tile([P, 4 * T_MAX], f32, tag=f"cf{q}")
            nc.sync.dma_start(out=gi[:], in_=gidx_d[q])
            nc.sync.dma_start(out=si[:], in_=sidx_d[q])
            nc.sync.dma_start(out=cf[:], in_=cofs_d[q])
            f0r = persist.tile([P, D], f32, tag=f"f0r{q}")
            g = persist.tile([P, D], f32, tag=f"g{q}")
            nc.gpsimd.indirect_dma_start(
                out=f0r[:], out_offset=None, in_=fsh[:],
                in_offset=bass.IndirectOffsetOnAxis(ap=gi[:, 0:1], axis=0))
            nc.scalar.copy(g[:], f0r[:])
            sc = small.tile([P, 6], f32, tag=f"sc{q}")
            for t in range(T_MAX):
                gsi = small.tile([P, 1], i32, tag=f"gsi{q}_{t}")
                nc.sync.dma_start(out=gsi[:], in_=gsix_d[q, t])
                gx = gxp.tile([P, D], f32, tag="gx")
                nc.gpsimd.indirect_dma_start(
                    out=gx[:], out_offset=None, in_=xin[:],
                    in_offset=bass.IndirectOffsetOnAxis(ap=gsi[:, 0:1], axis=0))
                # normalize gathered x rows, scale by (1-m) (0 for inactive rows)
                nc.scalar.activation(out=scr[:], in_=gx[:],
                                     func=ACTF.Square, accum_out=sc[:, 0:1])
                nc.scalar.sqrt(sc[:, 1:2], sc[:, 0:1])
                nc.vector.tensor_scalar_add(sc[:, 1:2], sc[:, 1:2], EPS)
                nc.vector.reciprocal(sc[:, 2:3], sc[:, 1:2])
                nc.vector.tensor_scalar(
                    out=sc[:, 3:4], in0=sc[:, 2:3], scalar1=cf[:, 4 * t + 1:4 * t + 2],
                    scalar2=None, op0=AL.mult)
                nc.vector.tensor_scalar_mul(gx[:], gx[:], sc[:, 3:4])
                # partial target logits: (0.8*xhat) . f_orig  (host rescales)
                nc.vector.tensor_mul(scr[:], gx[:], f0r[:])
                nc.vector.tensor_reduce(
                    st[:, q * T_MAX + t: q * T_MAX + t + 1], scr[:],
                    axis=AX.X, op=AL.add)
                # g = a*g + gx ; renorm (masked via coefficients)
                nc.vector.scalar_tensor_tensor(
                    out=g[:], in0=g[:], scalar=cf[:, 4 * t:4 * t + 1], in1=gx[:],
                    op0=AL.mult, op1=AL.add)
                nc.scalar.activation(out=scr[:], in_=g[:],
                                     func=ACTF.Square, accum_out=sc[:, 0:1])
                nc.scalar.sqrt(sc[:, 1:2], sc[:, 0:1])
                nc.vector.tensor_scalar(
                    out=sc[:, 4:5], in0=sc[:, 1:2], scalar1=EPS,
                    scalar2=cf[:, 4 * t + 2:4 * t + 3], op0=AL.add, op1=AL.mult)
                nc.vector.tensor_scalar_add(sc[:, 4:5], sc[:, 4:5], cf[:, 4 * t + 3:4 * t + 4])
                nc.vector.reciprocal(sc[:, 5:6], sc[:, 4:5])
                nc.vector.tensor_scalar_mul(g[:], g[:], sc[:, 5:6])
            gtiles.append(g)
            sitiles.append(si)

        # ---- Stage D: stream the shard: load -> PE transpose -> matmul ->
        #      exp+rowsum -> pass-through store ----
        for b, bsubs in enumerate(blocks):
            BR = sum(nr for _, nr in bsubs)
            r0 = bsubs[0][0]
            regular = len(bsubs) == SUB and all(nr == P for _, nr in bsubs)
            nat = natp.tile([P, SUB, D], f32, tag="nat")
            if regular:
                nc.sync.dma_start(
                    out=nat[:], in_=fsh[r0:r0 + SUB * P].rearrange("(a p) d -> p a d", p=P))
            else:
                for j, (rs, nr) in enumerate(bsubs):
                    nc.sync.dma_start(out=nat[:nr, j, :], in_=fsh[rs:rs + nr])
            rt = rtp.tile([P, 16 * SUB * P], f32, tag="rt")
            for k2 in range(16):
                pst = psTp.tile([P, 512], f32, tag="psT")
                cum = 0
                for j, (rs, nr) in enumerate(bsubs):
                    nc.tensor.transpose(
                        out=pst[:, cum:cum + nr],
                        in_=nat[:nr, j, k2 * P:(k2 + 1) * P],
                        identity=ident[:nr, :nr])
                    cum += nr
                if k2 % 2:
                    nc.scalar.copy(rt[:, k2 * BR:(k2 + 1) * BR].bitcast(f32r), pst[:, :BR])
                else:
                    nc.vector.tensor_copy(rt[:, k2 * BR:(k2 + 1) * BR].bitcast(f32r), pst[:, :BR])
            for m in range(2):
                psm = psMp.tile([P, 512], f32, tag="psM")
                for k2 in range(16):
                    nc.tensor.matmul(
                        psm[:, :BR],
                        lhsT=xT[:, k2 * B + m * P: k2 * B + (m + 1) * P].bitcast(f32r),
                        rhs=rt[:, k2 * BR:(k2 + 1) * BR].bitcast(f32r),
                        start=(k2 == 0), stop=(k2 == 15))
                esc = escp.tile([P, 512], f32, tag="esc")
                nc.scalar.activation(
                    out=esc[:, :BR], in_=psm[:, :BR], func=ACTF.Exp,
                    scale=ITEMP, accum_out=sums[:, m * NB + b: m * NB + b + 1])
            if regular:
                nc.sync.dma_start(
                    out=obank[r0:r0 + SUB * P].rearrange("(a p) d -> p a d", p=P), in_=nat[:])
            else:
                for j, (rs, nr) in enumerate(bsubs):
                    nc.sync.dma_start(out=obank[rs:rs + nr], in_=nat[:nr, j, :])

        # ---- Stage E: scatter updated rows (ordered after pass-through
        #      stores via DRAM byte-range WAW deps), finalize stats ----
        for q in range(RQ if VAR == "full" else 0):
            nc.gpsimd.indirect_dma_start(
                out=obank[:], out_offset=bass.IndirectOffsetOnAxis(ap=sitiles[q][:, 0:1], axis=0),
                in_=gtiles[q][:], in_offset=None,
                bounds_check=NS - 1, oob_is_err=False)
        for m in range(2):
            nc.vector.tensor_reduce(
                out=st[:, RQ * T_MAX + m: RQ * T_MAX + m + 1],
                in_=sums[:, m * NB:(m + 1) * NB], axis=AX.X, op=AL.add)
        with tc.tile_pool(name="psS", bufs=1, space="PSUM") as psSp, \
             tc.tile_pool(name="stT", bufs=1) as stTp:
            pss = psSp.tile([ST, P], f32)
            nc.tensor.transpose(
                out=pss[:], in_=st[:], identity=ident[:].bitcast(f32))
            stt = stTp.tile([ST, P], f32)
            nc.vector.tensor_copy(stt[:], pss[:])
            nc.sync.dma_start(out=ostat[:], in_=stt[:])

    nc.compile()
    return nc


def _make_in_maps(x, feats, percore, T_MAX, RQ):
    in_maps, unpack = [], []
    for c in range(NCORES):
        chains = percore[c]
        gidx = np.zeros((RQ, P, 1), np.int32)
        sidx = np.full((RQ, P, 1), NS, np.int32)  # NS = out-of-bounds -> skipped
        gsix = np.zeros((RQ, T_MAX, P, 1), np.int32)
        cofs = np.zeros((RQ, P, 4 * T_MAX), np.float32)
        for t in range(T_MAX):
            cofs[:, :, 4 * t + 0] = 1.0  # a: keep g
            cofs[:, :, 4 * t + 3] = 1.0  # 1-m: divisor 1
        upk = []
        for idx, (row, samps) in enumerate(chains):
            q, r = divmod(idx, P)
            gidx[q, r, 0] = row
            sidx[q, r, 0] = row
            for t, s in enumerate(samps):
                gsix[q, t, r, 0] = s
                cofs[q, r, 4 * t + 0] = MOM
                cofs[q, r, 4 * t + 1] = OMM
                cofs[q, r, 4 * t + 2] = 1.0
                cofs[q, r, 4 * t + 3] = 0.0
                upk.append((q, r, t, s))
        unpack.append(upk)
        in_maps.append({
            "xin": x,
            "fshard": np.ascontiguousarray(feats[c * NS:(c + 1) * NS]),
            "gidx": gidx, "sidx": sidx, "gsidx": gsix, "coefs": cofs,
        })
    return in_maps, unpack


def kernel(inputs, features, targets):
    x = np.ascontiguousarray(np.asarray(inputs, dtype=np.float32))
    feats = np.asarray(features, dtype=np.float32)
    tgt = np.asarray(targets).astype(np.int64)

    percore, T_MAX, RQ = _schedule(tgt)
    nc = _build(T_MAX, RQ)
    in_maps, unpack = _make_in_maps(x, feats, percore, T_MAX, RQ)

    from concourse.bass_utils import run_bass_kernel_spmd
    trace = bool(int(os.environ.get("KERNEL_TRACE", "0")))
    res = run_bass_kernel_spmd(nc, in_maps, list(range(NCORES)), trace=trace)
    _LAST_RESULTS["res"] = res
    if trace and res.exec_time_ns is not None:
        print(f"HW exec time: {res.exec_time_ns} ns")

    out = np.empty((N, D), np.float32)
    s_tot = np.zeros(B, np.float64)
    tl = np.zeros(B, np.float64)
    for c in range(NCORES):
        r = res.results[c]
        out[c * NS:(c + 1) * NS] = r["out_bank"]
        stats = r["out_stats"].astype(np.float64)
        s_tot += np.concatenate([stats[RQ * T_MAX], stats[RQ * T_MAX + 1]])
        for (q, rr, t, s) in unpack[c]:
            tl[s] = stats[q * T_MAX + t, rr] * (ITEMP / OMM)
    loss = np.float32(np.mean(np.log(s_tot) - tl))
    return loss, out


# revision 10
# speedup vs baseline: 1.0862x; 1.0862x over previous
"""Trainium2 (Bass/Tile) kernel for the ClusterMemory problem.

Computes, for full inputs:
  x = normalize(inputs)                      [B, D]
  logits = x @ features.T / TEMP             [B, N]
  loss = cross_entropy(logits, targets)      scalar
  new_features = sequential per-sample momentum update + renorm of bank rows

Sharding: the feature bank [N, D] is sharded row-wise across 8 NeuronCores.
Each core streams its shard once (HBM read), computes partial logits via the
tensor engine (on-chip PE transpose of the shard, fp32r matmuls), reduces
exp(logits) row-wise on the scalar engine, and passes the shard through to the
output bank.  The per-target momentum update (a scatter over <=256 rows) is
done on the owning core with indirect-DMA gathers of the affected rows +
masked chain updates (handles duplicate targets sequentially), then an
indirect-DMA scatter into the output bank, ordered after the pass-through
writes.  The host combines per-core partial softmax statistics (sum-exp and
target logits) into the scalar loss and concatenates the output shards.
"""

import os
import numpy as np

B, D, N = 256, 2048, 30000
NCORES = 8
NS = N // NCORES  # 3750 rows per core
TEMP = 0.05
ITEMP = 1.0 / TEMP
MOM = 0.2
OMM = 1.0 - MOM
EPS = 1e-12
P = 128
SUB = 3  # 128-row subtiles per streaming block

_LAST_RESULTS = {}


def _schedule(targets):
    """Per-core chains: for core c, sorted list of (local_row, [sample indices])."""
    percore = []
    t_max, r_max = 1, 1
    tl = [int(t) for t in targets]
    for c in range(NCORES):
        lo = c * NS
        rows = {}
        for i, t in enumerate(tl):
            if lo <= t < lo + NS:
                rows.setdefault(t - lo, []).append(i)
        chains = sorted(rows.items())
        percore.append(chains)
        if chains:
            t_max = max(t_max, max(len(s) for _, s in chains))
            r_max = max(r_max, len(chains))
    rq = max(1, -(-r_max // P))
    return percore, t_max, rq


def _build(T_MAX, RQ):
    VAR = os.environ.get("KVAR", "full")
    import concourse.bass as bass
    import concourse.tile as tile
    from concourse import bacc, mybir
    from concourse.masks import make_identity

    f32 = mybir.dt.float32
    f32r = mybir.dt.float32r
    i32 = mybir.dt.int32
    AL = mybir.AluOpType
    ACTF = mybir.ActivationFunctionType
    AX = mybir.AxisListType

    nc = bacc.Bacc(None, target_bir_lowering=False, debug=True)
    ST = RQ * T_MAX + 2

    xin = nc.dram_tensor("xin", [B, D], f32, kind="ExternalInput")
    fsh = nc.dram_tensor("fshard", [NS, D], f32, kind="ExternalInput")
    gidx_d = nc.dram_tensor("gidx", [RQ, P, 1], i32, kind="ExternalInput")
    sidx_d = nc.dram_tensor("sidx", [RQ, P, 1], i32, kind="ExternalInput")
    gsix_d = nc.dram_tensor("gsidx", [RQ, T_MAX, P, 1], i32, kind="ExternalInput")
    cofs_d = nc.dram_tensor("coefs", [RQ, P, 4 * T_MAX], f32, kind="ExternalInput")

    obank = nc.dram_tensor("out_bank", [NS, D], f32, kind="ExternalOutput")
    ostat = nc.dram_tensor("out_stats", [ST, P], f32, kind="ExternalOutput")

    # streaming structure: 29 full 128-row subtiles + one 38-row tail
    nfull = NS // P
    rem = NS - nfull * P
    subs = [(j * P, P) for j in range(nfull)]
    if rem:
        subs.append((nfull * P, rem))
    if VAR == "notail":
        subs = subs[:28]
    blocks = [subs[i:i + SUB] for i in range(0, len(subs), SUB)]
    if len(blocks) > 1 and any(nr != P for _, nr in blocks[-1]) :
        blocks = blocks[-1:] + blocks[:-1]
    NB = len(blocks)

    with tile.TileContext(nc) as tc, \
         tc.tile_pool(name="const", bufs=1) as constp, \
         tc.tile_pool(name="persist", bufs=1) as persist, \
         tc.tile_pool(name="small", bufs=1) as small, \
         tc.tile_pool(name="gx", bufs=2) as gxp, \
         tc.tile_pool(name="nat", bufs=3) as natp, \
         tc.tile_pool(name="rhsT", bufs=2) as rtp, \
         tc.tile_pool(name="psT", bufs=2, space="PSUM") as psTp, \
         tc.tile_pool(name="psM", bufs=2, space="PSUM") as psMp, \
         tc.tile_pool(name="esc", bufs=2) as escp:

        ident = constp.tile([P, P], f32)
        make_identity(nc, ident)

        xT = persist.tile([P, 16 * B], f32)      # [d(128), k2*256 + sample]
        sums = persist.tile([P, 2 * NB], f32)    # per-block exp-sums
        st = persist.tile([P, ST], f32)          # stats: tlogits + sumexp halves
        if VAR in ("noc", "notail"):
            nc.vector.memset(st[:], 0.0)
        scr = persist.tile([P, D], f32)          # elementwise scratch

        # ---- Stage A/B: normalize x, transpose to xT ----
        with tc.tile_pool(name="xtmp", bufs=1) as xtmp, \
             tc.tile_pool(name="psX", bufs=2, space="PSUM") as psXp:
            xraw = xtmp.tile([P, 2, D], f32)
            nc.sync.dma_start(out=xraw[:], in_=xin[:].rearrange("(k p) d -> p k d", p=P))
            ss = small.tile([P, 4], f32, tag="ssA")
            for k in range(2):
                nc.scalar.activation(out=scr[:], in_=xraw[:, k, :],
                                     func=ACTF.Square, accum_out=ss[:, 0:1])
                nc.scalar.sqrt(ss[:, 1:2], ss[:, 0:1])
                nc.vector.tensor_scalar_add(ss[:, 1:2], ss[:, 1:2], EPS)
                nc.vector.reciprocal(ss[:, 2:3], ss[:, 1:2])
                nc.vector.tensor_scalar_mul(xraw[:, k, :], xraw[:, k, :], ss[:, 2:3])
            for k2 in range(16):
                psx = psXp.tile([P, B], f32, tag="psX")
                for k in range(2):
                    nc.tensor.transpose(
                        out=psx[:, k * P:(k + 1) * P],
                        in_=xraw[:, k, k2 * P:(k2 + 1) * P],
                        identity=ident[:])
                if k2 % 2:
                    nc.scalar.copy(xT[:, k2 * B:(k2 + 1) * B].bitcast(f32r), psx[:])
                else:
                    nc.vector.tensor_copy(xT[:, k2 * B:(k2 + 1) * B].bitcast(f32r), psx[:])

        # ---- Stage C: gather target rows, chained momentum update ----
        gtiles, sitiles = [], []
        for q in range(RQ if VAR not in ("noc", "notail") else 0):
            gi = small.tile([P, 1], i32, tag=f"gi{q}")
            si = small.tile([P, 1], i32, tag=f"si{q}")
            cf = small.tile([P, 4 * T_MAX], f32, tag=f"cf{q}")
            nc.sync.dma_start(out=gi[:], in_=gidx_d[q])
            nc.sync.dma_start(out=si[:], in_=sidx_d[q])
            nc.sync.dma_start(out=cf[:], in_=cofs_d[q])
            f0r = persist.tile([P, D], f32, tag=f"f0r{q}")
            g = persist.tile([P, D], f32, tag=f"g{q}")
            nc.gpsimd.indirect_dma_start(
                out=f0r[:], out_offset=None, in_=fsh[:],
                in_offset=bass.IndirectOffsetOnAxis(ap=gi[:, 0:1], axis=0))
            nc.scalar.copy(g[:], f0r[:])
            sc = small.tile([P, 6], f32, tag=f"sc{q}")
            for t in range(T_MAX):
                gsi = small.tile([P, 1], i32, tag=f"gsi{q}_{t}")
                nc.sync.dma_start(out=gsi[:], in_=gsix_d[q, t])
                gx = gxp.tile([P, D], f32, tag="gx")
                nc.gpsimd.indirect_dma_start(
                    out=gx[:], out_offset=None, in_=xin[:],
                    in_offset=bass.IndirectOffsetOnAxis(ap=gsi[:, 0:1], axis=0))
                # normalize gathered x rows, scale by (1-m) (0 for inactive rows)
                nc.scalar.activation(out=scr[:], in_=gx[:],
                                     func=ACTF.Square, accum_out=sc[:, 0:1])
                nc.scalar.sqrt(sc[:, 1:2], sc[:, 0:1])
                nc.vector.tensor_scalar_add(sc[:, 1:2], sc[:, 1:2], EPS)
                nc.vector.reciprocal(sc[:, 2:3], sc[:, 1:2])
                nc.vector.tensor_scalar(
                    out=sc[:, 3:4], in0=sc[:, 2:3], scalar1=cf[:, 4 * t + 1:4 * t + 2],
                    scalar2=None, op0=AL.mult)
                nc.vector.tensor_scalar_mul(gx[:], gx[:], sc[:, 3:4])
                # partial target logits: (0.8*xhat) . f_orig  (host rescales)
                nc.vector.tensor_mul(scr[:], gx[:], f0r[:])
                nc.vector.tensor_reduce(
                    st[:, q * T_MAX + t: q * T_MAX + t + 1], scr[:],
                    axis=AX.X, op=AL.add)
                # g = a*g + gx ; renorm (masked via coefficients)
                nc.vector.scalar_tensor_tensor(
                    out=g[:], in0=g[:], scalar=cf[:, 4 * t:4 * t + 1], in1=gx[:],
                    op0=AL.mult, op1=AL.add)
                nc.scalar.activation(out=scr[:], in_=g[:],
                                     func=ACTF.Square, accum_out=sc[:, 0:1])
                nc.scalar.sqrt(sc[:, 1:2], sc[:, 0:1])
                nc.vector.tensor_scalar(
                    out=sc[:, 4:5], in0=sc[:, 1:2], scalar1=EPS,
                    scalar2=cf[:, 4 * t + 2:4 * t + 3], op0=AL.add, op1=AL.mult)
                nc.vector.tensor_scalar_add(sc[:, 4:5], sc[:, 4:5], cf[:, 4 * t + 3:4 * t + 4])
                nc.vector.reciprocal(sc[:, 5:6], sc[:, 4:5])
                nc.vector.tensor_scalar_mul(g[:], g[:], sc[:, 5:6])
            gtiles.append(g)
            sitiles.append(si)

        # ---- Stage D: stream the shard: load -> PE transpose -> matmul ->
        #      exp+rowsum -> pass-through store ----
        for b, bsubs in enumerate(blocks):
            BR = sum(nr for _, nr in bsubs)
            r0 = bsubs[0][0]
            regular = len(bsubs) == SUB and all(nr == P for _, nr in bsubs)
            nat = natp.tile([P, SUB, D], f32, tag="nat")
            if regular:
                nc.sync.dma_start(
                    out=nat[:], in_=fsh[r0:r0 + SUB * P].rearrange("(a p) d -> p a d", p=P))
            else:
                for j, (rs, nr) in enumerate(bsubs):
                    nc.sync.dma_start(out=nat[:nr, j, :], in_=fsh[rs:rs + nr])
            rt = rtp.tile([P, 16 * SUB * P], f32, tag="rt")
            for k2 in range(16):
                pst = psTp.tile([P, 512], f32, tag="psT")
                cum = 0
                for j, (rs, nr) in enumerate(bsubs):
                    nc.tensor.transpose(
                        out=pst[:, cum:cum + nr],
                        in_=nat[:nr, j, k2 * P:(k2 + 1) * P],
                        identity=ident[:nr, :nr])
                    cum += nr
                if k2 % 2:
                    nc.scalar.copy(rt[:, k2 * BR:(k2 + 1) * BR].bitcast(f32r), pst[:, :BR])
                else:
                    nc.vector.tensor_copy(rt[:, k2 * BR:(k2 + 1) * BR].bitcast(f32r), pst[:, :BR])
            for m in range(2):
                psm = psMp.tile([P, 512], f32, tag="psM")
                for k2 in range(16):
                    nc.tensor.matmul(
                        psm[:, :BR],
                        lhsT=xT[:, k2 * B + m * P: k2 * B + (m + 1) * P].bitcast(f32r),
                        rhs=rt[:, k2 * BR:(k2 + 1) * BR].bitcast(f32r),
                        start=(k2 == 0), stop=(k2 == 15))
                esc = escp.tile([P, 512], f32, tag="esc")
                nc.scalar.activation(
                    out=esc[:, :BR], in_=psm[:, :BR], func=ACTF.Exp,
                    scale=ITEMP, accum_out=sums[:, m * NB + b: m * NB + b + 1])
            if regular:
                nc.sync.dma_start(
                    out=obank[r0:r0 + SUB * P].rearrange("(a p) d -> p a d", p=P), in_=nat[:])
            else:
                for j, (rs, nr) in enumerate(bsubs):
                    nc.sync.dma_start(out=obank[rs:rs + nr], in_=nat[:nr, j, :])

        # ---- Stage E: scatter updated rows (ordered after pass-through
        #      stores via DRAM byte-range WAW deps), finalize stats ----
        for q in range(RQ if VAR == "full" else 0):
            nc.gpsimd.indirect_dma_start(
                out=obank[:], out_offset=bass.IndirectOffsetOnAxis(ap=sitiles[q][:, 0:1], axis=0),
                in_=gtiles[q][:], in_offset=None,
                bounds_check=NS - 1, oob_is_err=False)
        for m in range(2):
            nc.vector.tensor_reduce(
                out=st[:, RQ * T_MAX + m: RQ * T_MAX + m + 1],
                in_=sums[:, m * NB:(m + 1) * NB], axis=AX.X, op=AL.add)
        with tc.tile_pool(name="psS", bufs=1, space="PSUM") as psSp, \
             tc.tile_pool(name="stT", bufs=1) as stTp:
            pss = psSp.tile([ST, P], f32)
            nc.tensor.transpose(
                out=pss[:], in_=st[:], identity=ident[:].bitcast(f32))
            stt = stTp.tile([ST, P], f32)
            nc.vector.tensor_copy(stt[:], pss[:])
            nc.sync.dma_start(out=ostat[:], in_=stt[:])

    nc.compile()
    return nc


def _make_in_maps(x, feats, percore, T_MAX, RQ):
    in_maps, unpack = [], []
    for c in range(NCORES):
        chains = percore[c]
        gidx = np.zeros((RQ, P, 1), np.int32)
        sidx = np.full((RQ, P, 1), NS, np.int32)  # NS = out-of-bounds -> skipped
        gsix = np.zeros((RQ, T_MAX, P, 1), np.int32)
        cofs = np.zeros((RQ, P, 4 * T_MAX), np.float32)
        for t in range(T_MAX):
            cofs[:, :, 4 * t + 0] = 1.0  # a: keep g
            cofs[:, :, 4 * t + 3] = 1.0  # 1-m: divisor 1
        upk = []
        for idx, (row, samps) in enumerate(chains):
            q, r = divmod(idx, P)
            gidx[q, r, 0] = row
            sidx[q, r, 0] = row
            for t, s in enumerate(samps):
                gsix[q, t, r, 0] = s
                cofs[q, r, 4 * t + 0] = MOM
                cofs[q, r, 4 * t + 1] = OMM
                cofs[q, r, 4 * t + 2] = 1.0
                cofs[q, r, 4 * t + 3] = 0.0
                upk.append((q, r, t, s))
        unpack.append(upk)
        in_maps.append({
            "xin": x,
            "fshard": np.ascontiguousarray(feats[c * NS:(c + 1) * NS]),
            "gidx": gidx, "sidx": sidx, "gsidx": gsix, "coefs": cofs,
        })
    return in_maps, unpack


def kernel(inputs, features, targets):
    x = np.ascontiguousarray(np.asarray(inputs, dtype=np.float32))
    feats = np.asarray(features, dtype=np.float32)
    tgt = np.asarray(targets).astype(np.int64)

    percore, T_MAX, RQ = _schedule(tgt)
    nc = _build(T_MAX, RQ)
    in_maps, unpack = _make_in_maps(x, feats, percore, T_MAX, RQ)

    from concourse.bass_utils import run_bass_kernel_spmd
    trace = bool(int(os.environ.get("KERNEL_TRACE", "0")))
    res = run_bass_kernel_spmd(nc, in_maps, list(range(NCORES)), trace=trace)
    _LAST_RESULTS["res"] = res
    if trace and res.exec_time_ns is not None:
        print(f"HW exec time: {res.exec_time_ns} ns")

    out = np.empty((N, D), np.float32)
    s_tot = np.zeros(B, np.float64)
    tl = np.zeros(B, np.float64)
    for c in range(NCORES):
        r = res.results[c]
        out[c * NS:(c + 1) * NS] = r["out_bank"]
        stats = r["out_stats"].astype(np.float64)
        s_tot += np.concatenate([stats[RQ * T_MAX], stats[RQ * T_MAX + 1]])
        for (q, rr, t, s) in unpack[c]:
            tl[s] = stats[q * T_MAX + t, rr] * (ITEMP / OMM)
    loss = np.float32(np.mean(np.log(s_tot) - tl))
    return loss, out
